# revision 46
# baseline (speedup 1.0000x reference)
# Transformer-XL style relative-position attention on 8 Trainium2 NeuronCores.
#
# Contract: kernel(**inputs) takes the FULL unsharded inputs and returns the
# FULL [8, 256, 1024] output. Internally shards data-parallel over batch:
# core b computes batch element b. No collectives needed.
#
# v2 design (vs the 170us baseline):
#  * Host prep: inputs are uploaded pre-transposed and cast to f16, and the
#    batch-independent positional keys RW = R@Wkr (only 257 rows are ever
#    used) are precomputed on the host. This removes all gpsimd cast-DMAs,
#    all cat/R transposes and the RW matmuls from the device, and halves the
#    HBM bytes of every weight/activation load.
#  * Attention scores are computed KEY-major (keys on partitions) so the
#    softmax probabilities land directly in the layout the AV matmul needs:
#    the 96 PE transposes + 96 psum->sbuf copies of the baseline disappear.
#  * rel_shift stays a DRAM shear: BD = (q+v) @ RW^T is written query-major
#    contiguous ([128, 257] rows at scratch[i, 256 + s]), and read back
#    key-major with the access pattern [[1,128],[128,4],[767,256]] which
#    realizes band[j', jt, i] = BD[i, (jt*128+j') - i].  Mask comes free:
#    out-of-band reads hit NEG-prefilled scratch columns.
#  * The band is accumulated into the term_a psum by an identity matmul
#    (PE), not a DVE add; exp reads psum directly and writes the f16
#    probability tiles the AV matmuls consume.
#  * Softmax row sums come from a ones-column appended to each val tile
#    (pav row 64); normalization is a [1,256] reciprocal + rank-1 ones
#    outer-product + one DVE multiply per head, fused into the psum->sbuf
#    copy of the attention output.
#  * All DMAs are batched (3/4-dim APs): one DMA per weight half, one
#    scratch write + one sheared read per head.

import numpy as np

import concourse.bass as bass
import concourse.mybir as mybir
import concourse.tile as tile
from concourse import bacc, bass_utils
from concourse.masks import make_identity
from concourse.tile import add_dep_helper
from contextlib import ExitStack

F32 = mybir.dt.float32
F16 = mybir.dt.float16
AF = mybir.ActivationFunctionType
OP = mybir.AluOpType

DIM = 1024
HEADS = 16
DHEAD = 64
B = 8
N = 256          # query tokens (x)
M = 256          # memory tokens (h)
T = M + N        # 512 keys
SCALE = DHEAD ** -0.5
NEG = -30000.0   # f16-representable; *0.125 still underflows exp
SW = 768         # scratch row width
NS = 257         # valid relative offsets s = j - i in [0, 256]
NHB = 16         # scratch buffers: one per head (no reuse, no WAR deps)


def build_kernel():
    nc = bacc.Bacc("TRN2", target_bir_lowering=False, debug=False)

    catt_d = nc.dram_tensor("catT", [DIM, T], F16, kind="ExternalInput")
    wq_d = nc.dram_tensor("wq", [DIM, DIM], F16, kind="ExternalInput")
    wk_d = nc.dram_tensor("wk", [DIM, DIM], F16, kind="ExternalInput")
    wv_d = nc.dram_tensor("wv", [DIM, DIM], F16, kind="ExternalInput")
    wo_d = nc.dram_tensor("wo", [DIM, DIM], F16, kind="ExternalInput")
    rwst_d = nc.dram_tensor("rwst", [DIM, 258], F16, kind="ExternalInput")
    uuvv_d = nc.dram_tensor("uuvv", [128, 2], F32, kind="ExternalInput")
    out_d = nc.dram_tensor("out", [N, DIM], F16, kind="ExternalOutput")
    scr_d = nc.dram_tensor("scr", [NHB, N, SW], F16)
    junk_d = nc.dram_tensor("warm_junk", [128, 512], F16)

    with tile.TileContext(nc) as tc, ExitStack() as ctx:
        _body(ctx, tc, catt_d, wq_d, wk_d, wv_d, wo_d, rwst_d, uuvv_d,
              out_d, scr_d, junk_d)

    nc.compile()
    return nc


def _body(ctx, tc, catt_d, wq_d, wk_d, wv_d, wo_d, rwst_d, uuvv_d, out_d,
          scr_d, junk_d):
    nc = tc.nc

    const = ctx.enter_context(tc.tile_pool(name="const", bufs=1))
    persist = ctx.enter_context(tc.tile_pool(name="persist", bufs=1))
    work = ctx.enter_context(tc.tile_pool(name="work", bufs=4))
    ps_m = ctx.enter_context(tc.tile_pool(name="ps_m", bufs=3, space="PSUM"))
    ps_a = ctx.enter_context(tc.tile_pool(name="ps_a", bufs=3, space="PSUM"))
    ps_v = ctx.enter_context(tc.tile_pool(name="ps_v", bufs=1, space="PSUM"))

    # ---------------- constants ----------------
    ident_h = const.tile([128, 128], F16, tag="identh", name="ident_h")
    make_identity(nc, ident_h)
    ones_row = const.tile([1, 64], F16, tag="ones", name="ones_row")
    nc.vector.memset(ones_row, 1.0)
    junk = const.tile([128, 512], F16, tag="junk", name="junk")
    nc.vector.memset(junk, 1.0)
    uuvv = const.tile([128, 2], F32, tag="uuvv", name="uuvv_sb")

    # ---------------- input loads (all HWDGE, few big DMAs) ----------------
    catt_sb = persist.tile([128, 8, T], F16, tag="catt", name="catt_sb")
    wq_sb = persist.tile([128, 8, DIM], F16, tag="wq", name="wq_sb")
    wk_sb = persist.tile([128, 8, DIM], F16, tag="wk", name="wk_sb")
    wv_sb = persist.tile([128, 8, DIM], F16, tag="wv", name="wv_sb")
    wo_sb = persist.tile([128, 8, DIM], F16, tag="wo", name="wo_sb")
    rwst_sb = persist.tile([128, 8, 258], F16, tag="rwst", name="rwst_sb")

    def load_rows(sb, dr, ncol, d0, d1):
        # sb[p, dt, c] = dr[dt*128 + p, c] for dt in [d0, d1)
        src = bass.AP(dr[:, 0:1].tensor, d0 * 128 * ncol,
                      [[ncol, 128], [128 * ncol, d1 - d0], [1, ncol]])
        return nc.sync.dma_start(out=sb[:, d0:d1, :], in_=src)

    nc.sync.dma_start(out=uuvv, in_=uuvv_d[:, :])
    load_rows(catt_sb, catt_d, T, 0, 4)
    load_rows(catt_sb, catt_d, T, 4, 8)
    load_rows(wq_sb, wq_d, DIM, 0, 4)
    load_rows(wq_sb, wq_d, DIM, 4, 8)
    load_rows(rwst_sb, rwst_d, 258, 0, 8)
    load_rows(wk_sb, wk_d, DIM, 0, 4)
    load_rows(wk_sb, wk_d, DIM, 4, 8)

    # NEG prefill of the scratch pad columns [129,256) and [513,640).
    neg_sb = const.tile([128, 2032], F16, tag="neg", name="neg_sb")
    nc.gpsimd.memset(neg_sb, NEG)
    zinit = []
    for zi in range(4):
        dst = bass.AP(scr_d[0][:, 0:1].tensor, zi * 4 * N * SW + 129,
                      [[N * SW, 4], [SW, N], [384, 2], [1, 127]])
        zinit.append(nc.sync.dma_start(out=dst, in_=neg_sb[:, 0:2032]))

    load_rows(wv_sb, wv_d, DIM, 0, 4)
    load_rows(wv_sb, wv_d, DIM, 4, 8)
    load_rows(wo_sb, wo_d, DIM, 0, 4)
    load_rows(wo_sb, wo_d, DIM, 4, 8)

    # ---------------- PE warm-up (p-state ramp + covers load latency) ------
    pwarm = ps_m.tile([128, 512], F32, tag="pm", name="ps_warm")
    for wi in range(16):
        nc.tensor.matmul(pwarm, junk[:, 0:128], junk,
                         start=(wi == 0), stop=(wi == 15))
    junk2 = const.tile([128, 512], F16, tag="junk2", name="junk2")
    nc.vector.tensor_copy(junk2, pwarm)
    nc.sync.dma_start(out=junk_d[:, :], in_=junk2)

    # ---------------- q projection (x tokens only) ----------------
    # quT/qvT[p, ft, i]: feature-major q with u/v folded in.
    quT = persist.tile([128, 8, N], F16, tag="quT", name="quT")
    qvT = persist.tile([128, 8, N], F16, tag="qvT", name="qvT")
    for ft in range(8):
        pq = ps_m.tile([128, 512], F32, tag="pm", name=f"ps_q{ft}")
        for dt in range(8):
            nc.tensor.matmul(pq[:, 0:N], wq_sb[:, dt, ft * 128:(ft + 1) * 128],
                             catt_sb[:, dt, M:T], start=(dt == 0), stop=(dt == 7))
        nc.vector.tensor_scalar_add(quT[:, ft, :], pq[:, 0:N], uuvv[:, 0:1])
        nc.vector.tensor_scalar_add(qvT[:, ft, :], pq[:, 0:N], uuvv[:, 1:2])

    # ---------------- k projection interleaved with BD + scratch ----------
    # kT[p, ft, j]: feature-major keys.
    # BD[i, s] = (q+v)[i] . RW[s]  (query-major), written to scratch rows.
    kT = persist.tile([128, 8, T], F16, tag="kT", name="kT")
    wr_insts = [None] * HEADS
    rd_insts = [None] * HEADS
    bands = [None] * HEADS

    def bd_head(hh):
        ft, ro = hh // 2, (hh % 2) * 64
        bsb = work.tile([128, 2, NS], F16, tag="bsb", name=f"bsb{hh}", bufs=4)
        for qb in range(2):
            pb = ps_m.tile([128, 512], F32, tag="pm", name=f"ps_b{hh}_{qb}")
            nc.tensor.matmul(pb[:, 0:NS],
                             qvT[ro:ro + 64, ft, qb * 128:(qb + 1) * 128],
                             rwst_sb[ro:ro + 64, ft, 0:NS],
                             start=True, stop=True)
            nc.scalar.copy(bsb[:, qb, :], pb[:, 0:NS])
        dst = bass.AP(scr_d[0][:, 0:1].tensor, hh * N * SW + 256,
                      [[SW, 128], [128 * SW, 2], [1, NS]])
        w = nc.gpsimd.dma_start(out=dst, in_=bsb[:, :, :])
        wr_insts[hh] = w
        band = work.tile([128, 2, 384], F16, tag="band", name=f"band{hh}",
                         bufs=NHB)
        src = bass.AP(scr_d[0][:, 0:1].tensor, hh * N * SW + 256,
                      [[SW - 1, 128], [128 * SW, 2], [1, 384]])
        r = nc.sync.dma_start(out=band[:, :, :], in_=src)
        add_dep_helper(r.ins, w.ins, sync=True, reason="scratch RAW")
        add_dep_helper(r.ins, zinit[hh // 4].ins, sync=True,
                       reason="pad RAW on mask-init")
        rd_insts[hh] = r
        bands[hh] = band

    for ft in range(8):
        pk = ps_m.tile([128, 512], F32, tag="pm", name=f"ps_k{ft}")
        for dt in range(8):
            nc.tensor.matmul(pk, wk_sb[:, dt, ft * 128:(ft + 1) * 128],
                             catt_sb[:, dt, :], start=(dt == 0), stop=(dt == 7))
        nc.vector.tensor_copy(kT[:, ft, :], pk)
        bd_head(2 * ft)
        bd_head(2 * ft + 1)

    # ---------------- v projection ----------------
    # val65[p, jt, h, 0:64] = token-major values; col 64 = ones (gives the
    # softmax row sums as row 64 of the AV psum).
    val65 = persist.tile([128, 4, HEADS, 65], F16, tag="val65", name="val65")
    for jt in range(4):
        for nh in range(2):
            pv = ps_m.tile([128, 512], F32, tag="pm", name=f"ps_v{jt}_{nh}")
            for dt in range(8):
                nc.tensor.matmul(pv, catt_sb[:, dt, jt * 128:(jt + 1) * 128],
                                 wv_sb[:, dt, nh * 512:(nh + 1) * 512],
                                 start=(dt == 0), stop=(dt == 7))
            nc.vector.tensor_copy(val65[:, jt, nh * 8:(nh + 1) * 8, 0:64], pv)
        nc.gpsimd.memset(val65[:, jt, :, 64:65], 1.0)

    # ---------------- attention ----------------
    # Scores stay query-major; probs of 4 heads at a time are transposed
    # key-major by one XBAR DMA-transpose, so AV output is query-major and
    # the softmax normalization folds into its per-partition psum->sbuf
    # scale.  AV runs one group behind the scores so the XBAR latency hides
    # behind the next group's score matmuls.
    aoT = persist.tile([128, 8, N], F16, tag="aoT", name="aoT")
    grp = {}

    def scores_head(hh):
        ft, ro = hh // 2, (hh % 2) * 64
        band = bands[hh]
        att = work.tile([128, 2, 384], F16, tag="att", name=f"att{hh}", bufs=4)
        for qb in range(2):
            pa = ps_a.tile([128, 384], F32, tag="pa", name=f"ps_a{hh}_{qb}")
            nc.tensor.matmul(pa, quT[ro:ro + 64, ft, qb * 128:(qb + 1) * 128],
                             kT[ro:ro + 64, ft, qb * 128:qb * 128 + 384],
                             start=True, stop=False)
            nc.tensor.matmul(pa, ident_h, band[:, qb, :],
                             start=False, stop=True)
            nc.scalar.activation(att[:, qb, :], pa, AF.Exp,
                                 bias=0.0, scale=SCALE)
        # attT[j', k, i'] = att[i', k // 3, (k % 3) * 128 + j']
        attT = work.tile([128, 6, 128], F16, tag="attT", name=f"attT{hh}",
                         bufs=4)
        nc.sync.dma_start(out=attT[:, :, :], in_=att[:, 0:2, :],
                          transpose=True)
        grp[hh] = attT

    def av_head(hh):
        ft, ro = hh // 2, (hh % 2) * 64
        attT = grp[hh]
        # pav rows 0:64 = head output (feat-major), row 64 = row sums S.
        pav = ps_v.tile([65, N], F32, tag="pav", name=f"ps_av{hh}", bufs=1)
        for qb in range(2):
            for w in range(3):
                nc.tensor.matmul(pav[:, qb * 128:(qb + 1) * 128],
                                 val65[:, qb + w, hh, 0:65],
                                 attT[:, qb * 3 + w, :],
                                 start=(w == 0), stop=(w == 2))
        rcps = work.tile([1, N], F16, tag="rcps", name=f"rcps{hh}", bufs=4)
        with nc.allow_low_precision(reason="1/S in f16 is plenty for 2e-2"):
            nc.vector.reciprocal(rcps, pav[64:65, :])
        pbc = ps_v.tile([64, N], F32, tag="pbc", name=f"ps_bc{hh}", bufs=1)
        nc.tensor.matmul(pbc, ones_row, rcps, start=True, stop=True)
        bcs = work.tile([64, N], F16, tag="bcs", name=f"bcs{hh}", bufs=4)
        nc.vector.tensor_copy(bcs, pbc)
        nc.vector.tensor_tensor(aoT[ro:ro + 64, ft, :], pav[0:64, :], bcs,
                                OP.mult)

    for hh in range(HEADS):
        scores_head(hh)
        av_head(hh)

    # ---------------- output projection ----------------
    osb = persist.tile([128, 2, DIM], F16, tag="osb", name="osb")
    for tt in range(2):
        for nh in range(2):
            po = ps_m.tile([128, 512], F32, tag="pm", name=f"ps_o{tt}_{nh}")
            for ft in range(8):
                nc.tensor.matmul(po, aoT[:, ft, tt * 128:(tt + 1) * 128],
                                 wo_sb[:, ft, nh * 512:(nh + 1) * 512],
                                 start=(ft == 0), stop=(ft == 7))
            nc.vector.tensor_copy(osb[:, tt, nh * 512:(nh + 1) * 512], po)
    dst = bass.AP(out_d[:, 0:1].tensor, 0, [[DIM, 128], [128 * DIM, 2], [1, DIM]])
    nc.sync.dma_start(out=dst, in_=osb[:, :, :])


def host_prep(inputs):
    x = np.asarray(inputs["x"], dtype=np.float32)
    h = np.asarray(inputs["h"], dtype=np.float32)
    wqkv = np.asarray(inputs["Wqkv"], dtype=np.float32)
    wkr = np.asarray(inputs["Wkr"], dtype=np.float32)
    r = np.asarray(inputs["R"], dtype=np.float32)
    u = np.asarray(inputs["u"], dtype=np.float32)
    v = np.asarray(inputs["v"], dtype=np.float32)
    wout = np.asarray(inputs["Wout"], dtype=np.float32)

    wq = np.ascontiguousarray(wqkv[:, 0:DIM].astype(np.float16))
    wk = np.ascontiguousarray(wqkv[:, DIM:2 * DIM].astype(np.float16))
    wv = np.ascontiguousarray(wqkv[:, 2 * DIM:3 * DIM].astype(np.float16))
    wo = np.ascontiguousarray(wout.astype(np.float16))

    # positional keys: only offsets s = j - i in [0, 256] are unmasked;
    # RW row for offset s is (R @ Wkr)[(s + 768) % 1024].
    rows = (np.arange(NS) + 768) % 1024
    rws = r[rows] @ wkr                        # [257, 1024] f32
    rwst = np.zeros((DIM, 258), dtype=np.float16)
    rwst[:, 0:NS] = rws.T.astype(np.float16)

    uuvv = np.stack([np.tile(u, 2), np.tile(v, 2)], axis=1)
    uuvv = np.ascontiguousarray(uuvv.astype(np.float32))

    catts = []
    for b in range(B):
        cat = np.concatenate([h[b], x[b]], axis=0)          # [512, 1024]
        catts.append(np.ascontiguousarray(cat.T.astype(np.float16)))

    shared = {"wq": wq, "wk": wk, "wv": wv, "wo": wo, "rwst": rwst,
              "uuvv": uuvv}
    return catts, shared


_NC_CACHE = {}


def _get_nc():
    if "nc" not in _NC_CACHE:
        _NC_CACHE["nc"] = build_kernel()
    return _NC_CACHE["nc"]


def _run(inputs, trace=False):
    catts, shared = host_prep(inputs)
    nc = _get_nc()
    in_maps = [dict(shared, catT=catts[b]) for b in range(B)]
    res = bass_utils.run_bass_kernel_spmd(
        nc, in_maps, core_ids=list(range(B)), trace=trace)
    out = np.stack([res.results[b]["out"].astype(np.float32)
                    for b in range(B)])
    return out, res


def kernel(**inputs):
    out, _ = _run(inputs, trace=False)
    return out


# revision 48
# speedup vs baseline: 1.1078x; 1.1078x over previous
# Transformer-XL style relative-position attention on 8 Trainium2 NeuronCores.
#
# Contract: kernel(**inputs) takes the FULL unsharded inputs and returns the
# FULL [8, 256, 1024] output. Internally shards data-parallel over batch:
# core b computes batch element b. No collectives needed.
#
# v2 design (vs the 170us baseline):
#  * Host prep: inputs are uploaded pre-transposed and cast to f16, and the
#    batch-independent positional keys RW = R@Wkr (only 257 rows are ever
#    used) are precomputed on the host. This removes all gpsimd cast-DMAs,
#    all cat/R transposes and the RW matmuls from the device, and halves the
#    HBM bytes of every weight/activation load.
#  * Attention scores are computed KEY-major (keys on partitions) so the
#    softmax probabilities land directly in the layout the AV matmul needs:
#    the 96 PE transposes + 96 psum->sbuf copies of the baseline disappear.
#  * rel_shift stays a DRAM shear: BD = (q+v) @ RW^T is written query-major
#    contiguous ([128, 257] rows at scratch[i, 256 + s]), and read back
#    key-major with the access pattern [[1,128],[128,4],[767,256]] which
#    realizes band[j', jt, i] = BD[i, (jt*128+j') - i].  Mask comes free:
#    out-of-band reads hit NEG-prefilled scratch columns.
#  * The band is accumulated into the term_a psum by an identity matmul
#    (PE), not a DVE add; exp reads psum directly and writes the f16
#    probability tiles the AV matmuls consume.
#  * Softmax row sums come from a ones-column appended to each val tile
#    (pav row 64); normalization is a [1,256] reciprocal + rank-1 ones
#    outer-product + one DVE multiply per head, fused into the psum->sbuf
#    copy of the attention output.
#  * All DMAs are batched (3/4-dim APs): one DMA per weight half, one
#    scratch write + one sheared read per head.

import numpy as np

import concourse.bass as bass
import concourse.mybir as mybir
import concourse.tile as tile
from concourse import bacc, bass_utils
from concourse.masks import make_identity
from concourse.tile import add_dep_helper
from contextlib import ExitStack

F32 = mybir.dt.float32
F16 = mybir.dt.float16
AF = mybir.ActivationFunctionType
OP = mybir.AluOpType

DIM = 1024
HEADS = 16
DHEAD = 64
B = 8
N = 256          # query tokens (x)
M = 256          # memory tokens (h)
T = M + N        # 512 keys
SCALE = DHEAD ** -0.5
NEG = -30000.0   # f16-representable; *0.125 still underflows exp
SW = 768         # scratch row width
NS = 257         # valid relative offsets s = j - i in [0, 256]
NHB = 16         # scratch buffers: one per head (no reuse, no WAR deps)


def build_kernel():
    nc = bacc.Bacc("TRN2", target_bir_lowering=False, debug=False)

    catt_d = nc.dram_tensor("catT", [DIM, T], F16, kind="ExternalInput")
    wq_d = nc.dram_tensor("wq", [DIM, DIM], F16, kind="ExternalInput")
    wk_d = nc.dram_tensor("wk", [DIM, DIM], F16, kind="ExternalInput")
    wv_d = nc.dram_tensor("wv", [DIM, DIM], F16, kind="ExternalInput")
    wo_d = nc.dram_tensor("wo", [DIM, DIM], F16, kind="ExternalInput")
    rwst_d = nc.dram_tensor("rwst", [DIM, 258], F16, kind="ExternalInput")
    uuvv_d = nc.dram_tensor("uuvv", [128, 2], F32, kind="ExternalInput")
    out_d = nc.dram_tensor("out", [N, DIM], F16, kind="ExternalOutput")
    scr_d = nc.dram_tensor("scr", [NHB, N, SW], F16)
    junk_d = nc.dram_tensor("warm_junk", [128, 512], F16)

    with tile.TileContext(nc) as tc, ExitStack() as ctx:
        _body(ctx, tc, catt_d, wq_d, wk_d, wv_d, wo_d, rwst_d, uuvv_d,
              out_d, scr_d, junk_d)

    nc.compile()
    return nc


def _body(ctx, tc, catt_d, wq_d, wk_d, wv_d, wo_d, rwst_d, uuvv_d, out_d,
          scr_d, junk_d):
    nc = tc.nc

    const = ctx.enter_context(tc.tile_pool(name="const", bufs=1))
    persist = ctx.enter_context(tc.tile_pool(name="persist", bufs=1))
    work = ctx.enter_context(tc.tile_pool(name="work", bufs=4))
    ps_m = ctx.enter_context(tc.tile_pool(name="ps_m", bufs=3, space="PSUM"))
    ps_a = ctx.enter_context(tc.tile_pool(name="ps_a", bufs=3, space="PSUM"))
    ps_v = ctx.enter_context(tc.tile_pool(name="ps_v", bufs=2, space="PSUM"))

    # ---------------- constants ----------------
    ident_h = const.tile([128, 128], F16, tag="identh", name="ident_h")
    make_identity(nc, ident_h)
    junk = const.tile([128, 512], F16, tag="junk", name="junk")
    nc.vector.memset(junk, 1.0)
    uuvv = const.tile([128, 2], F32, tag="uuvv", name="uuvv_sb")

    # ---------------- input loads (all HWDGE, few big DMAs) ----------------
    catt_sb = persist.tile([128, 8, T], F16, tag="catt", name="catt_sb")
    wq_sb = persist.tile([128, 8, DIM], F16, tag="wq", name="wq_sb")
    wk_sb = persist.tile([128, 8, DIM], F16, tag="wk", name="wk_sb")
    wv_sb = persist.tile([128, 8, DIM], F16, tag="wv", name="wv_sb")
    wo_sb = persist.tile([128, 8, DIM], F16, tag="wo", name="wo_sb")
    rwst_sb = persist.tile([128, 8, 258], F16, tag="rwst", name="rwst_sb")

    def load_rows(sb, dr, ncol, d0, d1):
        # sb[p, dt, c] = dr[dt*128 + p, c] for dt in [d0, d1)
        src = bass.AP(dr[:, 0:1].tensor, d0 * 128 * ncol,
                      [[ncol, 128], [128 * ncol, d1 - d0], [1, ncol]])
        return nc.sync.dma_start(out=sb[:, d0:d1, :], in_=src)

    nc.sync.dma_start(out=uuvv, in_=uuvv_d[:, :])
    load_rows(catt_sb, catt_d, T, 0, 4)
    load_rows(catt_sb, catt_d, T, 4, 8)
    load_rows(wq_sb, wq_d, DIM, 0, 4)
    load_rows(wq_sb, wq_d, DIM, 4, 8)
    load_rows(rwst_sb, rwst_d, 258, 0, 8)
    load_rows(wk_sb, wk_d, DIM, 0, 4)
    load_rows(wk_sb, wk_d, DIM, 4, 8)

    # NEG prefill of the scratch pad columns [129,256) and [513,640).
    neg_sb = const.tile([128, 2032], F16, tag="neg", name="neg_sb")
    nc.gpsimd.memset(neg_sb, NEG)
    zinit = []
    for zi in range(4):
        dst = bass.AP(scr_d[0][:, 0:1].tensor, zi * 4 * N * SW + 129,
                      [[N * SW, 4], [SW, N], [384, 2], [1, 127]])
        zinit.append(nc.sync.dma_start(out=dst, in_=neg_sb[:, 0:2032]))

    load_rows(wv_sb, wv_d, DIM, 0, 4)
    load_rows(wv_sb, wv_d, DIM, 4, 8)
    load_rows(wo_sb, wo_d, DIM, 0, 4)
    load_rows(wo_sb, wo_d, DIM, 4, 8)

    # ---------------- PE warm-up (p-state ramp + covers load latency) ------
    pwarm = ps_m.tile([128, 512], F32, tag="pm", name="ps_warm")
    for wi in range(16):
        nc.tensor.matmul(pwarm, junk[:, 0:128], junk,
                         start=(wi == 0), stop=(wi == 15))
    junk2 = const.tile([128, 512], F16, tag="junk2", name="junk2")
    nc.vector.tensor_copy(junk2, pwarm)
    nc.sync.dma_start(out=junk_d[:, :], in_=junk2)

    # ---------------- q projection (x tokens only) ----------------
    # quT/qvT[p, ft, i]: feature-major q with u/v folded in.
    quT = persist.tile([128, 8, N], F16, tag="quT", name="quT")
    qvT = persist.tile([128, 8, N], F16, tag="qvT", name="qvT")
    for ft in range(8):
        pq = ps_m.tile([128, 512], F32, tag="pm", name=f"ps_q{ft}")
        for dt in range(8):
            nc.tensor.matmul(pq[:, 0:N], wq_sb[:, dt, ft * 128:(ft + 1) * 128],
                             catt_sb[:, dt, M:T], start=(dt == 0), stop=(dt == 7))
        nc.vector.tensor_scalar_add(quT[:, ft, :], pq[:, 0:N], uuvv[:, 0:1])
        nc.vector.tensor_scalar_add(qvT[:, ft, :], pq[:, 0:N], uuvv[:, 1:2])

    # ---------------- k projection interleaved with BD + scratch ----------
    # kT[p, ft, j]: feature-major keys.
    # BD[i, s] = (q+v)[i] . RW[s]  (query-major), written to scratch rows.
    kT = persist.tile([128, 8, T], F16, tag="kT", name="kT")
    wr_insts = [None] * HEADS
    rd_insts = [None] * HEADS
    bands = [None] * HEADS

    def bd_head(hh):
        ft, ro = hh // 2, (hh % 2) * 64
        bsb = work.tile([128, 2, NS], F16, tag="bsb", name=f"bsb{hh}", bufs=4)
        for qb in range(2):
            pb = ps_m.tile([128, 512], F32, tag="pm", name=f"ps_b{hh}_{qb}")
            nc.tensor.matmul(pb[:, 0:NS],
                             qvT[ro:ro + 64, ft, qb * 128:(qb + 1) * 128],
                             rwst_sb[ro:ro + 64, ft, 0:NS],
                             start=True, stop=True)
            nc.scalar.copy(bsb[:, qb, :], pb[:, 0:NS])
        dst = bass.AP(scr_d[0][:, 0:1].tensor, hh * N * SW + 256,
                      [[SW, 128], [128 * SW, 2], [1, NS]])
        w = nc.gpsimd.dma_start(out=dst, in_=bsb[:, :, :])
        wr_insts[hh] = w
        band = work.tile([128, 2, 384], F16, tag="band", name=f"band{hh}",
                         bufs=NHB)
        src = bass.AP(scr_d[0][:, 0:1].tensor, hh * N * SW + 256,
                      [[SW - 1, 128], [128 * SW, 2], [1, 384]])
        r = nc.sync.dma_start(out=band[:, :, :], in_=src)
        add_dep_helper(r.ins, w.ins, sync=True, reason="scratch RAW")
        add_dep_helper(r.ins, zinit[hh // 4].ins, sync=True,
                       reason="pad RAW on mask-init")
        rd_insts[hh] = r
        bands[hh] = band

    for ft in range(8):
        pk = ps_m.tile([128, 512], F32, tag="pm", name=f"ps_k{ft}")
        for dt in range(8):
            nc.tensor.matmul(pk, wk_sb[:, dt, ft * 128:(ft + 1) * 128],
                             catt_sb[:, dt, :], start=(dt == 0), stop=(dt == 7))
        nc.vector.tensor_copy(kT[:, ft, :], pk)
        bd_head(2 * ft)
        bd_head(2 * ft + 1)

    # ---------------- v projection ----------------
    # val[p, jt, h, d] = token-major values.
    val = persist.tile([128, 4, HEADS, DHEAD], F16, tag="val", name="val")
    for jt in range(4):
        for nh in range(2):
            pv = ps_m.tile([128, 512], F32, tag="pm", name=f"ps_v{jt}_{nh}")
            for dt in range(8):
                nc.tensor.matmul(pv, catt_sb[:, dt, jt * 128:(jt + 1) * 128],
                                 wv_sb[:, dt, nh * 512:(nh + 1) * 512],
                                 start=(dt == 0), stop=(dt == 7))
            nc.vector.tensor_copy(val[:, jt, nh * 8:(nh + 1) * 8, :], pv)

    # ---------------- attention ----------------
    # Scores stay query-major; probs of 4 heads at a time are transposed
    # key-major by one XBAR DMA-transpose, so AV output is query-major and
    # the softmax normalization folds into its per-partition psum->sbuf
    # scale.  AV runs one group behind the scores so the XBAR latency hides
    # behind the next group's score matmuls.
    aoQ = persist.tile([128, 2, DIM], F16, tag="aoQ", name="aoQ")
    grp = {}

    def scores_head(hh):
        ft, ro = hh // 2, (hh % 2) * 64
        band = bands[hh]
        att = work.tile([128, 2, 384], F16, tag="att", name=f"att{hh}", bufs=4)
        rcp = work.tile([128, 2], F32, tag="rcp", name=f"rcp{hh}", bufs=4)
        for qb in range(2):
            pa = ps_a.tile([128, 384], F32, tag="pa", name=f"ps_a{hh}_{qb}")
            nc.tensor.matmul(pa, quT[ro:ro + 64, ft, qb * 128:(qb + 1) * 128],
                             kT[ro:ro + 64, ft, qb * 128:qb * 128 + 384],
                             start=True, stop=False)
            nc.tensor.matmul(pa, ident_h, band[:, qb, :],
                             start=False, stop=True)
            ssum = work.tile([128, 1], F32, tag="ssum", name=f"ss{hh}_{qb}",
                             bufs=12)
            nc.scalar.activation(att[:, qb, :], pa, AF.Exp,
                                 bias=0.0, scale=SCALE, accum_out=ssum)
            nc.vector.reciprocal(rcp[:, qb:qb + 1], ssum)
        # attT[j', k, i'] = att[i', k // 3, (k % 3) * 128 + j']
        attT = work.tile([128, 6, 128], F16, tag="attT", name=f"attT{hh}",
                         bufs=4)
        eng = nc.scalar if hh % 2 == 0 else nc.sync
        eng.dma_start(out=attT[:, :, :], in_=att[:, 0:2, :], transpose=True)
        grp[hh] = (attT, rcp)

    def av_head(hh):
        attT, rcp = grp[hh]
        for qb in range(2):
            pav = ps_v.tile([128, DHEAD], F32, tag="pav",
                            name=f"ps_av{hh}_{qb}")
            for w in range(3):
                nc.tensor.matmul(pav, attT[:, qb * 3 + w, :],
                                 val[:, qb + w, hh, :],
                                 start=(w == 0), stop=(w == 2))
            nc.vector.tensor_scalar_mul(
                aoQ[:, qb, hh * DHEAD:(hh + 1) * DHEAD], pav,
                rcp[:, qb:qb + 1])

    for hh in range(HEADS):
        scores_head(hh)
        av_head(hh)

    # ---------------- output projection ----------------
    osb = persist.tile([128, 2, DIM], F16, tag="osb", name="osb")
    aoTs = []
    for tt in range(2):
        aoT = work.tile([128, 8, 128], F16, tag="aoT", name=f"aoT{tt}", bufs=2)
        nc.sync.dma_start(out=aoT[:, :, :], in_=aoQ[:, tt, :], transpose=True)
        aoTs.append(aoT)
    for tt in range(2):
        aoT = aoTs[tt]
        for nh in range(2):
            po = ps_m.tile([128, 512], F32, tag="pm", name=f"ps_o{tt}_{nh}")
            for ft in range(8):
                nc.tensor.matmul(po, aoT[:, ft, :],
                                 wo_sb[:, ft, nh * 512:(nh + 1) * 512],
                                 start=(ft == 0), stop=(ft == 7))
            nc.vector.tensor_copy(osb[:, tt, nh * 512:(nh + 1) * 512], po)
    dst = bass.AP(out_d[:, 0:1].tensor, 0, [[DIM, 128], [128 * DIM, 2], [1, DIM]])
    nc.sync.dma_start(out=dst, in_=osb[:, :, :])


def host_prep(inputs):
    x = np.asarray(inputs["x"], dtype=np.float32)
    h = np.asarray(inputs["h"], dtype=np.float32)
    wqkv = np.asarray(inputs["Wqkv"], dtype=np.float32)
    wkr = np.asarray(inputs["Wkr"], dtype=np.float32)
    r = np.asarray(inputs["R"], dtype=np.float32)
    u = np.asarray(inputs["u"], dtype=np.float32)
    v = np.asarray(inputs["v"], dtype=np.float32)
    wout = np.asarray(inputs["Wout"], dtype=np.float32)

    wq = np.ascontiguousarray(wqkv[:, 0:DIM].astype(np.float16))
    wk = np.ascontiguousarray(wqkv[:, DIM:2 * DIM].astype(np.float16))
    wv = np.ascontiguousarray(wqkv[:, 2 * DIM:3 * DIM].astype(np.float16))
    wo = np.ascontiguousarray(wout.astype(np.float16))

    # positional keys: only offsets s = j - i in [0, 256] are unmasked;
    # RW row for offset s is (R @ Wkr)[(s + 768) % 1024].
    rows = (np.arange(NS) + 768) % 1024
    rws = r[rows] @ wkr                        # [257, 1024] f32
    rwst = np.zeros((DIM, 258), dtype=np.float16)
    rwst[:, 0:NS] = rws.T.astype(np.float16)

    uuvv = np.stack([np.tile(u, 2), np.tile(v, 2)], axis=1)
    uuvv = np.ascontiguousarray(uuvv.astype(np.float32))

    catts = []
    for b in range(B):
        cat = np.concatenate([h[b], x[b]], axis=0)          # [512, 1024]
        catts.append(np.ascontiguousarray(cat.T.astype(np.float16)))

    shared = {"wq": wq, "wk": wk, "wv": wv, "wo": wo, "rwst": rwst,
              "uuvv": uuvv}
    return catts, shared


_NC_CACHE = {}


def _get_nc():
    if "nc" not in _NC_CACHE:
        _NC_CACHE["nc"] = build_kernel()
    return _NC_CACHE["nc"]


def _run(inputs, trace=False):
    catts, shared = host_prep(inputs)
    nc = _get_nc()
    in_maps = [dict(shared, catT=catts[b]) for b in range(B)]
    res = bass_utils.run_bass_kernel_spmd(
        nc, in_maps, core_ids=list(range(B)), trace=trace)
    out = np.stack([res.results[b]["out"].astype(np.float32)
                    for b in range(B)])
    return out, res


def kernel(**inputs):
    out, _ = _run(inputs, trace=False)
    return out
